# revision 25
# baseline (speedup 1.0000x reference)
"""AxialAttention (width=False, no positional) — Bass/Tile kernel on 8 trn2 cores.

Sharding: data-parallel over N (one image per core). All BN affine work is
folded into the weight matrices on the host:

  - qkv BN scale folds into w_qkv rows; q/k biases reduce (after softmax
    shift-invariance) to one extra contraction channel: q~ = [q_raw; 1],
    k^ = [sc_g*sk*Kraw; sc_g*(bq.sk)^T Wk x]; sim BN scale sc_g folds into k^.
  - v bias + out BN fold into the V weights / a host-side final bias.

Device program per core (bf16 SBUF, f32 PSUM):
  x (C=128, (w,h)) --K/QE/QO matmuls--> staging qst/qb-parity + kst strips;
  small SBUF->SBUF DMAs restage each group-pair's 9 contraction channels to
  partition base 0 in per-pair column blocks (q128/k128, zero rows are
  memset once per buffer slot on GPSIMD).  Per w: 4 full-128-row matmuls
  (one per group pair, serial, no PE tiling) give S^T (j, g*128+i) in one
  2-bank PSUM tile; one ACT exp -> PT bf16; 8 AV matmuls (lhsT=PT_g,
  rhs=vT_g|ones col) -> (i, g*17+c) with col 16 = softmax denominator.
  vT comes from per-w matmuls with x_w stationary.  Host divides by the
  denominator, adds the final bias, transposes to (op, H, W).
"""

import numpy as np
import ml_dtypes

import concourse.bacc as bacc
import concourse.mybir as mybir
import concourse.tile as tile
from concourse.bass_utils import run_bass_kernel_spmd

BF16 = mybir.dt.bfloat16
F16 = mybir.dt.float16
F32 = mybir.dt.float32
NPBF16 = ml_dtypes.bfloat16
NPF16 = np.float16

N, C, H, W = 8, 128, 128, 128
G = 8
EPS = 1e-5
CHUNK = 16                       # w-columns per pipeline chunk
NCHUNK = W // CHUNK
EXP_SHIFT = -25.0                # uniform exp shift, cancels in num/den

_NC = None


def _build_nc():
    nc = bacc.Bacc()

    x_d = nc.dram_tensor("x", [C, W * H], F16, kind="ExternalInput")
    qones_d = nc.dram_tensor("qones", [1, CHUNK * 128], F16, kind="ExternalInput")
    wqe_d = nc.dram_tensor("wqe", [C, 128], F16, kind="ExternalInput")
    wqo_d = nc.dram_tensor("wqo", [C, 128], F16, kind="ExternalInput")
    wk_d = nc.dram_tensor("wk", [C, 128], F16, kind="ExternalInput")
    wv_d = nc.dram_tensor("wv", [C, 128], F16, kind="ExternalInput")
    out_d = nc.dram_tensor("out", [H, W * 136], BF16, kind="ExternalOutput")

    with tile.TileContext(nc) as tc:
        with (
            tc.tile_pool(name="wp", bufs=1) as wp,
            tc.tile_pool(name="xp", bufs=2) as xp,
            tc.tile_pool(name="qst", bufs=2) as qstp,
            tc.tile_pool(name="kst", bufs=2) as kstp,
            tc.tile_pool(name="q128", bufs=2) as q128p,
            tc.tile_pool(name="k128", bufs=2) as k128p,
            tc.tile_pool(name="vt", bufs=2) as vtp,
            tc.tile_pool(name="ot", bufs=2) as otp,
            tc.tile_pool(name="pt", bufs=3) as ptp,
            tc.tile_pool(name="psA", bufs=2, space="PSUM") as psA,
            tc.tile_pool(name="psB", bufs=4, space="PSUM") as psB,
        ):
            bias_t = wp.tile([128, 1], F32, tag="ebias")
            nc.vector.memset(bias_t, EXP_SHIFT)
            wqe_t = wp.tile([C, 128], F16, tag="wqe")
            wqo_t = wp.tile([C, 128], F16, tag="wqo")
            wk_t = wp.tile([C, 128], F16, tag="wk")
            wv_t = wp.tile([C, 128], F16, tag="wv")
            nc.sync.dma_start(out=wqe_t, in_=wqe_d[:, :])
            nc.sync.dma_start(out=wqo_t, in_=wqo_d[:, :])
            nc.sync.dma_start(out=wk_t, in_=wk_d[:, :])
            nc.sync.dma_start(out=wv_t, in_=wv_d[:, :])

            for ch in range(NCHUNK):
                x_t = xp.tile([C, CHUNK * 128], F16, tag="x")
                nc.sync.dma_start(
                    out=x_t, in_=x_d[:, ch * CHUNK * 128:(ch + 1) * CHUNK * 128]
                )

                qst_t = qstp.tile([128, CHUNK * 256], F16, tag="qst")
                kst_t = kstp.tile([128, CHUNK * 128], F16, tag="kst")
                q128_t = q128p.tile([128, CHUNK * 1024], F16, tag="q128")
                k128_t = k128p.tile([128, CHUNK * 512], F16, tag="k128")
                vt_t = vtp.tile([128, CHUNK * 136], BF16, tag="vt")
                ot_t = otp.tile([128, CHUNK * 136], BF16, tag="ot")

                # zero-fill the restaged tiles once per buffer slot; later
                # chunks reuse the slots and only rewrite the data rows.
                if ch < 2:
                    nc.gpsimd.memset(q128_t, 0.0)
                    nc.gpsimd.memset(k128_t, 0.0)

                # ---- QKV matmuls over 512-wide (4 w) sub-blocks ----
                for s in range(CHUNK // 4):
                    rhs = x_t[:, s * 512:(s + 1) * 512]

                    pk = psB.tile([128, 512], F32, tag="ps512")
                    nc.tensor.matmul(pk, lhsT=wk_t, rhs=rhs, start=True, stop=True)
                    nc.vector.tensor_copy(kst_t[:, s * 512:(s + 1) * 512], pk)

                    qst_blk = qst_t[:, s * 1024:(s + 1) * 1024].rearrange(
                        "p (w t i) -> p w t i", t=2, i=128
                    )
                    pe = psB.tile([128, 512], F32, tag="ps512")
                    nc.tensor.matmul(pe, lhsT=wqe_t, rhs=rhs, start=True, stop=True)
                    nc.vector.tensor_copy(
                        qst_blk[:, :, 0, :],
                        pe.rearrange("p (w i) -> p w i", i=128),
                    )
                    po = psB.tile([128, 512], F32, tag="ps512")
                    nc.tensor.matmul(po, lhsT=wqo_t, rhs=rhs, start=True, stop=True)
                    nc.vector.tensor_copy(
                        qst_blk[:, :, 1, :],
                        po.rearrange("p (w i) -> p w i", i=128),
                    )

                    # restage the 32-row strips to partition base 0 into
                    # per-pair column blocks (zero rows persist from the
                    # slot memset).
                    kst_blk = kst_t[:, s * 512:(s + 1) * 512].rearrange(
                        "p (w j) -> p w j", j=128
                    )
                    k128_blk = k128_t[:, s * 2048:(s + 1) * 2048].rearrange(
                        "p (w t j) -> p w t j", t=4, j=128
                    )
                    qst_blk = qst_t[:, s * 1024:(s + 1) * 1024].rearrange(
                        "p (w par i) -> p w par i", par=2, i=128
                    )
                    q128_blk = q128_t[:, s * 4096:(s + 1) * 4096].rearrange(
                        "p (w t par i) -> p w t par i", t=4, par=2, i=128
                    )
                    for t in range(4):
                        nc.sync.dma_start(
                            out=k128_blk[0:25, :, t, :],
                            in_=kst_blk[32 * t:32 * t + 25, :, :],
                        )
                        nc.sync.dma_start(
                            out=q128_blk[0:8, :, t, 0, :],
                            in_=qst_blk[32 * t:32 * t + 8, :, 0, :],
                        )
                        nc.sync.dma_start(
                            out=q128_blk[16:24, :, t, 1, :],
                            in_=qst_blk[32 * t + 16:32 * t + 24, :, 1, :],
                        )

                # ones rows of q~ via DMA (row 8 even halves / row 24 odd)
                q128_f = q128_t.rearrange(
                    "p (b par i) -> p b par i", par=2, i=128
                )
                ones_in = qones_d[0:1, 0:128].rearrange(
                    "p (a i) -> p a i", a=1
                ).to_broadcast((1, CHUNK * 4, 128))
                nc.sync.dma_start(out=q128_f[8:9, :, 0, :], in_=ones_in)
                nc.sync.dma_start(out=q128_f[24:25, :, 1, :], in_=ones_in)

                # ---- vT matmuls (x_w stationary) ----
                for wl in range(CHUNK):
                    pv = psB.tile([128, 512], F32, tag="ps512")
                    nc.tensor.matmul(
                        pv[:, 0:128],
                        lhsT=x_t[:, wl * 128:(wl + 1) * 128],
                        rhs=wv_t,
                        start=True, stop=True,
                    )
                    nc.vector.tensor_copy(
                        vt_t.rearrange("p (w g c) -> p w g c", g=G, c=17)[
                            :, wl, :, 0:16
                        ],
                        pv[:, 0:128].rearrange("p (g c) -> p g c", c=16),
                    )
                # softmax-denominator ones columns
                nc.vector.memset(
                    vt_t.rearrange("p (w g c) -> p w g c", g=G, c=17)[:, :, :, 16],
                    1.0,
                )

                # ---- attention per w ----
                for wl in range(CHUNK):
                    sp = psA.tile([128, 1024], F32, tag="spsum")
                    for t in range(4):
                        nc.tensor.matmul(
                            sp[:, t * 256:(t + 1) * 256],
                            lhsT=k128_t[:, wl * 512 + t * 128:wl * 512 + (t + 1) * 128],
                            rhs=q128_t[:, wl * 1024 + t * 256:wl * 1024 + (t + 1) * 256],
                            start=True, stop=True,
                        )
                    ptt = ptp.tile([128, 1024], BF16, tag="pt")
                    nc.scalar.activation(
                        ptt, sp, mybir.ActivationFunctionType.Exp, bias=bias_t[:, 0:1]
                    )

                    o2 = psB.tile([128, 512], F32, tag="ps512")
                    for g in range(G):
                        nc.tensor.matmul(
                            o2[:, g * 17:(g + 1) * 17],
                            lhsT=ptt[:, g * 128:(g + 1) * 128],
                            rhs=vt_t[:, wl * 136 + g * 17:wl * 136 + (g + 1) * 17],
                            start=True, stop=True,
                        )
                    nc.vector.tensor_copy(
                        ot_t[:, wl * 136:(wl + 1) * 136], o2[:, 0:136]
                    )

                nc.sync.dma_start(
                    out=out_d[:, ch * CHUNK * 136:(ch + 1) * CHUNK * 136],
                    in_=ot_t,
                )

    return nc


def _get_nc():
    global _NC
    if _NC is None:
        _NC = _build_nc()
        if not _NC.is_finalized():
            _NC.finalize()
    return _NC


def _host_prep(x, w_qkv, qkv_gamma, qkv_beta, qkv_mean, qkv_var,
               sim_gamma, sim_var, out_gamma, out_beta, out_mean, out_var):
    s = (qkv_gamma / np.sqrt(qkv_var + EPS)).astype(np.float64)
    b = qkv_beta - qkv_mean * s
    Ws = (w_qkv.astype(np.float64) * s[:, None]).reshape(G, 32, C)
    bs = b.reshape(G, 32)
    Wq, Wk, Wv = Ws[:, 0:8], Ws[:, 8:16], Ws[:, 16:32]
    bq, bv = bs[:, 0:8], bs[:, 16:32]

    sc = sim_gamma / np.sqrt(sim_var + EPS)                     # (G,)
    s2 = out_gamma / np.sqrt(out_var + EPS)                     # (128,) op
    b2 = out_beta - out_mean * s2
    bfinal = (b2 + s2 * bv.reshape(128)).astype(np.float32)

    WQE = np.zeros((128, C), np.float64)
    WQO = np.zeros((128, C), np.float64)
    WKm = np.zeros((128, C), np.float64)
    WVm = np.zeros((128, C), np.float64)
    for t in range(4):
        WQE[32 * t:32 * t + 8] = Wq[2 * t]
        WQO[32 * t + 16:32 * t + 24] = Wq[2 * t + 1]
    for g in range(G):
        WKm[16 * g:16 * g + 8] = sc[g] * Wk[g]
        WKm[16 * g + 8] = sc[g] * (bq[g] @ Wk[g])
        WVm[16 * g:16 * g + 16] = s2[16 * g:16 * g + 16, None] * Wv[g]

    wqe = np.ascontiguousarray(WQE.T).astype(NPF16)             # (C, 128)
    wqo = np.ascontiguousarray(WQO.T).astype(NPF16)
    wk = np.ascontiguousarray(WKm.T).astype(NPF16)
    wv = np.ascontiguousarray(WVm.T).astype(NPF16)

    # x: (N, C, H, W) -> per core (C, W, H) bf16
    xb = np.ascontiguousarray(x.transpose(0, 1, 3, 2)).astype(NPF16)
    xb = xb.reshape(N, C, W * H)
    return xb, wqe, wqo, wk, wv, bfinal


def _run(inputs, trace=False):
    nc = _get_nc()
    xb, wqe, wqo, wk, wv, bfinal = _host_prep(
        inputs["x"], inputs["w_qkv"],
        inputs["qkv_gamma"], inputs["qkv_beta"], inputs["qkv_mean"],
        inputs["qkv_var"], inputs["sim_gamma"], inputs["sim_var"],
        inputs["out_gamma"], inputs["out_beta"], inputs["out_mean"],
        inputs["out_var"],
    )
    qones = np.ones((1, CHUNK * 128), NPF16)
    in_maps = [
        {"x": xb[n], "wqe": wqe, "wqo": wqo, "wk": wk, "wv": wv, "qones": qones}
        for n in range(N)
    ]
    res = run_bass_kernel_spmd(
        nc, in_maps, core_ids=list(range(N)), trace=trace
    )

    outs = np.empty((N, 128, H, W), np.float32)
    for n in range(N):
        raw = res.results[n]["out"].astype(np.float32)
        raw = raw.reshape(H, W, G, 17)
        o = raw[..., :16] / raw[..., 16:17]                     # (H, W, G, 16)
        outs[n] = o.transpose(2, 3, 0, 1).reshape(128, H, W)
    outs += bfinal[None, :, None, None]
    return outs, res


def kernel(x, w_qkv, qkv_gamma, qkv_beta, qkv_mean, qkv_var,
           sim_gamma, sim_beta, sim_mean, sim_var,
           out_gamma, out_beta, out_mean, out_var):
    inputs = dict(
        x=np.asarray(x, np.float32), w_qkv=np.asarray(w_qkv, np.float32),
        qkv_gamma=np.asarray(qkv_gamma, np.float32),
        qkv_beta=np.asarray(qkv_beta, np.float32),
        qkv_mean=np.asarray(qkv_mean, np.float32),
        qkv_var=np.asarray(qkv_var, np.float32),
        sim_gamma=np.asarray(sim_gamma, np.float32),
        sim_var=np.asarray(sim_var, np.float32),
        out_gamma=np.asarray(out_gamma, np.float32),
        out_beta=np.asarray(out_beta, np.float32),
        out_mean=np.asarray(out_mean, np.float32),
        out_var=np.asarray(out_var, np.float32),
    )
    outs, _ = _run(inputs, trace=False)
    return outs


# revision 37
# speedup vs baseline: 8359.8061x; 8359.8061x over previous
"""AxialAttention (width=False, no positional) — Bass/Tile kernel on 8 trn2 cores.

Sharding: data-parallel over N (one image per core). All BN affine work is
folded into the weight matrices on the host:

  - qkv BN scale folds into w_qkv rows; q/k biases reduce (after softmax
    shift-invariance) to one extra contraction channel: q~ = [q_raw; 1],
    k^ = [sc_g*sk*Kraw; sc_g*(bq.sk)^T Wk x]; sim BN scale sc_g folds into k^.
  - v bias + out BN fold into the V weights / a host-side final bias.

Device program per core (bf16 SBUF, f32 PSUM):
  x (C=128, (w,h)) --K/QE/QO matmuls--> staging qst/qb-parity + kst strips;
  small SBUF->SBUF DMAs restage each group-pair's 9 contraction channels to
  partition base 0 in per-pair column blocks (q128/k128, zero rows are
  memset once per buffer slot on GPSIMD).  Per w: 4 full-128-row matmuls
  (one per group pair, serial, no PE tiling) give S^T (j, g*128+i) in one
  2-bank PSUM tile; one ACT exp -> PT bf16; 8 AV matmuls (lhsT=PT_g,
  rhs=vT_g|ones col) -> (i, g*17+c) with col 16 = softmax denominator.
  vT comes from per-w matmuls with x_w stationary.  Host divides by the
  denominator, adds the final bias, transposes to (op, H, W).
"""

import numpy as np
import ml_dtypes

import concourse.bacc as bacc
import concourse.mybir as mybir
import concourse.tile as tile
from concourse.bass_utils import run_bass_kernel_spmd

BF16 = mybir.dt.bfloat16
F16 = mybir.dt.float16
F32 = mybir.dt.float32
NPBF16 = ml_dtypes.bfloat16
NPF16 = np.float16

N, C, H, W = 8, 128, 128, 128
G = 8
EPS = 1e-5
CHUNK = 8                        # w-columns per pipeline chunk
NCHUNK = W // CHUNK
EXP_SHIFT = -25.0                # uniform exp shift, cancels in num/den

_NC = None
_NC_LOOP = {}


def _build_nc(loop_n=None):
    nc = bacc.Bacc()

    x_d = nc.dram_tensor("x", [C, W * H], F16, kind="ExternalInput")
    qones_d = nc.dram_tensor("qones", [1, CHUNK * 128], F16, kind="ExternalInput")
    wqe_d = nc.dram_tensor("wqe", [C, 128], F16, kind="ExternalInput")
    wqo_d = nc.dram_tensor("wqo", [C, 128], F16, kind="ExternalInput")
    wk_d = nc.dram_tensor("wk", [C, 128], F16, kind="ExternalInput")
    wv_d = nc.dram_tensor("wv", [C, 128], F16, kind="ExternalInput")
    out_d = nc.dram_tensor("out", [H, W * 136], BF16, kind="ExternalOutput")

    with tile.TileContext(nc) as tc:
        with (
            tc.tile_pool(name="wp", bufs=1) as wp,
            tc.tile_pool(name="xp", bufs=3) as xp,
            tc.tile_pool(name="qst", bufs=3) as qstp,
            tc.tile_pool(name="kst", bufs=3) as kstp,
            tc.tile_pool(name="q128", bufs=3) as q128p,
            tc.tile_pool(name="k128", bufs=3) as k128p,
            tc.tile_pool(name="vt", bufs=3) as vtp,
            tc.tile_pool(name="ot", bufs=2) as otp,
            tc.tile_pool(name="pt", bufs=3) as ptp,
            tc.tile_pool(name="psA", bufs=2, space="PSUM") as psA,
            tc.tile_pool(name="psB", bufs=3, space="PSUM") as psB,
            tc.tile_pool(name="psC", bufs=1, space="PSUM") as psC,
        ):
            bias_t = wp.tile([128, 1], F32, tag="ebias")
            nc.vector.memset(bias_t, EXP_SHIFT)
            wqe_t = wp.tile([C, 128], F16, tag="wqe")
            wqo_t = wp.tile([C, 128], F16, tag="wqo")
            wk_t = wp.tile([C, 128], F16, tag="wk")
            wv_t = wp.tile([C, 128], F16, tag="wv")
            nc.sync.dma_start(out=wqe_t, in_=wqe_d[:, :])
            nc.sync.dma_start(out=wqo_t, in_=wqo_d[:, :])
            nc.sync.dma_start(out=wk_t, in_=wk_d[:, :])
            nc.sync.dma_start(out=wv_t, in_=wv_d[:, :])

            def prep_gen(ch, slot):
                """Generator emitting prep work in small quanta so it can be
                interleaved into the previous chunk's attention stream."""
                ctx = tc.high_priority(offset=2000)
                ctx.__enter__()
                x_t = xp.tile([C, CHUNK * 128], F16, tag="x")
                nc.gpsimd.dma_start(
                    out=x_t, in_=x_d[:, ch * CHUNK * 128:(ch + 1) * CHUNK * 128]
                )

                qst_t = qstp.tile([128, CHUNK * 256], F16, tag="qst")
                kst_t = kstp.tile([128, CHUNK * 128], F16, tag="kst")
                q128_t = q128p.tile([128, CHUNK * 1024], F16, tag="q128")
                k128_t = k128p.tile([128, CHUNK * 512], F16, tag="k128")
                vt_t = vtp.tile([128, CHUNK * 136], BF16, tag="vt")
                slot[ch] = (q128_t, k128_t, vt_t)
                yield

                # QKV matmuls over 512-wide (4 w) sub-blocks
                for s in range(CHUNK // 4):
                    rhs = x_t[:, s * 512:(s + 1) * 512]

                    pk = psB.tile([128, 512], F32, tag="ps512")
                    nc.tensor.matmul(pk, lhsT=wk_t, rhs=rhs, start=True, stop=True)
                    nc.vector.tensor_copy(kst_t[:, s * 512:(s + 1) * 512], pk)
                    yield

                    qst_blk = qst_t[:, s * 1024:(s + 1) * 1024].rearrange(
                        "p (w t i) -> p w t i", t=2, i=128
                    )
                    pe = psB.tile([128, 512], F32, tag="ps512")
                    nc.tensor.matmul(pe, lhsT=wqe_t, rhs=rhs, start=True, stop=True)
                    nc.vector.tensor_copy(
                        qst_blk[:, :, 0, :],
                        pe.rearrange("p (w i) -> p w i", i=128),
                    )
                    yield
                    po = psB.tile([128, 512], F32, tag="ps512")
                    nc.tensor.matmul(po, lhsT=wqo_t, rhs=rhs, start=True, stop=True)
                    nc.vector.tensor_copy(
                        qst_blk[:, :, 1, :],
                        po.rearrange("p (w i) -> p w i", i=128),
                    )
                    yield

                # restage each pair's 32-row strip to partition base 0 as a
                # t-major column block — one fully contiguous DMA per pair.
                for t in range(4):
                    nc.sync.dma_start(
                        out=k128_t[0:32, t * CHUNK * 128:(t + 1) * CHUNK * 128],
                        in_=kst_t[32 * t:32 * t + 32, :],
                    )
                    nc.sync.dma_start(
                        out=q128_t[0:32, t * CHUNK * 256:(t + 1) * CHUNK * 256],
                        in_=qst_t[32 * t:32 * t + 32, :],
                    )
                    yield

                # ones rows of q~ via DMA (row 8 even halves / row 24 odd)
                q128_f = q128_t.rearrange(
                    "p (b par i) -> p b par i", par=2, i=128
                )
                ones_in = qones_d[0:1, 0:128].rearrange(
                    "p (a i) -> p a i", a=1
                ).to_broadcast((1, CHUNK * 4, 128))
                nc.sync.dma_start(out=q128_f[8:9, :, 0, :], in_=ones_in)
                nc.sync.dma_start(out=q128_f[24:25, :, 1, :], in_=ones_in)
                yield

                # vT matmuls (x_w stationary), 4 w per PSUM tile
                for s in range(CHUNK // 4):
                    pv = psB.tile([128, 512], F32, tag="ps512")
                    for v in range(4):
                        wl = s * 4 + v
                        nc.tensor.matmul(
                            pv[:, v * 128:(v + 1) * 128],
                            lhsT=x_t[:, wl * 128:(wl + 1) * 128],
                            rhs=wv_t,
                            start=True, stop=True,
                        )
                    nc.vector.tensor_copy(
                        vt_t.rearrange("p (w g c) -> p w g c", g=G, c=17)[
                            :, s * 4:(s + 1) * 4, :, 0:16
                        ],
                        pv.rearrange("p (w g c) -> p w g c", g=G, c=16),
                    )
                    yield
                nc.vector.memset(
                    vt_t.rearrange("p (w g c) -> p w g c", g=G, c=17)[:, :, :, 16],
                    1.0,
                )
                ctx.__exit__(None, None, None)

            def pull(gen, n=1):
                if gen is None:
                    return
                for _ in range(n):
                    try:
                        next(gen)
                    except StopIteration:
                        return

            def attention(ch, state, gen):
                q128_t, k128_t, vt_t = state
                ot_t = otp.tile([128, CHUNK * 136], BF16, tag="ot")

                def emit_av(ptt, wl):
                    o2 = psC.tile([128, 136], F32, tag="o2")
                    for g in range(G):
                        nc.tensor.matmul(
                            o2[:, g * 17:(g + 1) * 17],
                            lhsT=ptt[:, g * 128:(g + 1) * 128],
                            rhs=vt_t[:, wl * 136 + g * 17:wl * 136 + (g + 1) * 17],
                            start=True, stop=True,
                        )
                    nc.vector.tensor_copy(
                        ot_t[:, wl * 136:(wl + 1) * 136], o2[:, 0:136]
                    )

                prev = None
                for wl in range(CHUNK):
                    sp = psA.tile([128, 1024], F32, tag="spsum")
                    for t in range(4):
                        nc.tensor.matmul(
                            sp[:, t * 256:(t + 1) * 256],
                            lhsT=k128_t[0:32, (t * CHUNK + wl) * 128:(t * CHUNK + wl + 1) * 128],
                            rhs=q128_t[0:32, (t * CHUNK + wl) * 256:(t * CHUNK + wl + 1) * 256],
                            start=True, stop=True,
                        )
                    ptt = ptp.tile([128, 1024], BF16, tag="pt")
                    nc.scalar.activation(
                        ptt, sp, mybir.ActivationFunctionType.Exp, bias=bias_t[:, 0:1]
                    )
                    if prev is not None:
                        emit_av(*prev)
                    prev = (ptt, wl)
                    pull(gen, 2)
                emit_av(*prev)

                nc.gpsimd.dma_start(
                    out=out_d[:, ch * CHUNK * 136:(ch + 1) * CHUNK * 136],
                    in_=ot_t,
                )

            # chunk-level software pipeline with fine-grained interleave of
            # the next chunk's prep into this chunk's attention stream.
            def pipeline():
                slot = {}
                g0 = prep_gen(0, slot)
                pull(g0, 100)
                for ch in range(NCHUNK):
                    gen = prep_gen(ch + 1, slot) if ch + 1 < NCHUNK else None
                    pull(gen, 1)
                    attention(ch, slot[ch], gen)
                    pull(gen, 100)

            if loop_n is None:
                pipeline()
            else:
                with tc.For_i(0, loop_n, 1):
                    pipeline()

    return nc


def _get_nc(loop_n=None):
    global _NC
    if loop_n is not None:
        if loop_n not in _NC_LOOP:
            nc = _build_nc(loop_n)
            if not nc.is_finalized():
                nc.finalize()
            _NC_LOOP[loop_n] = nc
        return _NC_LOOP[loop_n]
    if _NC is None:
        _NC = _build_nc()
        if not _NC.is_finalized():
            _NC.finalize()
    return _NC


def _host_prep(x, w_qkv, qkv_gamma, qkv_beta, qkv_mean, qkv_var,
               sim_gamma, sim_var, out_gamma, out_beta, out_mean, out_var):
    s = (qkv_gamma / np.sqrt(qkv_var + EPS)).astype(np.float64)
    b = qkv_beta - qkv_mean * s
    Ws = (w_qkv.astype(np.float64) * s[:, None]).reshape(G, 32, C)
    bs = b.reshape(G, 32)
    Wq, Wk, Wv = Ws[:, 0:8], Ws[:, 8:16], Ws[:, 16:32]
    bq, bv = bs[:, 0:8], bs[:, 16:32]

    sc = sim_gamma / np.sqrt(sim_var + EPS)                     # (G,)
    s2 = out_gamma / np.sqrt(out_var + EPS)                     # (128,) op
    b2 = out_beta - out_mean * s2
    bfinal = (b2 + s2 * bv.reshape(128)).astype(np.float32)

    WQE = np.zeros((128, C), np.float64)
    WQO = np.zeros((128, C), np.float64)
    WKm = np.zeros((128, C), np.float64)
    WVm = np.zeros((128, C), np.float64)
    for t in range(4):
        WQE[32 * t:32 * t + 8] = Wq[2 * t]
        WQO[32 * t + 16:32 * t + 24] = Wq[2 * t + 1]
    for g in range(G):
        WKm[16 * g:16 * g + 8] = sc[g] * Wk[g]
        WKm[16 * g + 8] = sc[g] * (bq[g] @ Wk[g])
        WVm[16 * g:16 * g + 16] = s2[16 * g:16 * g + 16, None] * Wv[g]

    wqe = np.ascontiguousarray(WQE.T).astype(NPF16)             # (C, 128)
    wqo = np.ascontiguousarray(WQO.T).astype(NPF16)
    wk = np.ascontiguousarray(WKm.T).astype(NPF16)
    wv = np.ascontiguousarray(WVm.T).astype(NPF16)

    # x: (N, C, H, W) -> per core (C, W, H) bf16
    xb = np.ascontiguousarray(x.transpose(0, 1, 3, 2)).astype(NPF16)
    xb = xb.reshape(N, C, W * H)
    return xb, wqe, wqo, wk, wv, bfinal


def _run(inputs, trace=False, loop_n=None):
    nc = _get_nc(loop_n)
    xb, wqe, wqo, wk, wv, bfinal = _host_prep(
        inputs["x"], inputs["w_qkv"],
        inputs["qkv_gamma"], inputs["qkv_beta"], inputs["qkv_mean"],
        inputs["qkv_var"], inputs["sim_gamma"], inputs["sim_var"],
        inputs["out_gamma"], inputs["out_beta"], inputs["out_mean"],
        inputs["out_var"],
    )
    qones = np.ones((1, CHUNK * 128), NPF16)
    in_maps = [
        {"x": xb[n], "wqe": wqe, "wqo": wqo, "wk": wk, "wv": wv, "qones": qones}
        for n in range(N)
    ]
    res = run_bass_kernel_spmd(
        nc, in_maps, core_ids=list(range(N)), trace=trace
    )

    outs = np.empty((N, 128, H, W), np.float32)
    for n in range(N):
        raw = res.results[n]["out"].astype(np.float32)
        raw = raw.reshape(H, W, G, 17)
        o = raw[..., :16] / raw[..., 16:17]                     # (H, W, G, 16)
        outs[n] = o.transpose(2, 3, 0, 1).reshape(128, H, W)
    outs += bfinal[None, :, None, None]
    return outs, res


def kernel(x, w_qkv, qkv_gamma, qkv_beta, qkv_mean, qkv_var,
           sim_gamma, sim_beta, sim_mean, sim_var,
           out_gamma, out_beta, out_mean, out_var):
    inputs = dict(
        x=np.asarray(x, np.float32), w_qkv=np.asarray(w_qkv, np.float32),
        qkv_gamma=np.asarray(qkv_gamma, np.float32),
        qkv_beta=np.asarray(qkv_beta, np.float32),
        qkv_mean=np.asarray(qkv_mean, np.float32),
        qkv_var=np.asarray(qkv_var, np.float32),
        sim_gamma=np.asarray(sim_gamma, np.float32),
        sim_var=np.asarray(sim_var, np.float32),
        out_gamma=np.asarray(out_gamma, np.float32),
        out_beta=np.asarray(out_beta, np.float32),
        out_mean=np.asarray(out_mean, np.float32),
        out_var=np.asarray(out_var, np.float32),
    )
    outs, _ = _run(inputs, trace=False)
    return outs


# revision 39
# speedup vs baseline: 14372.7364x; 1.7193x over previous
"""AxialAttention (width=False, no positional) — Bass/Tile kernel on 8 trn2 cores.

Sharding: data-parallel over N (one image per core). All BN affine work is
folded into the weight matrices on the host:

  - qkv BN scale folds into w_qkv rows; q/k biases reduce (after softmax
    shift-invariance) to one extra contraction channel: q~ = [q_raw; 1],
    k^ = [sc_g*sk*Kraw; sc_g*(bq.sk)^T Wk x]; sim BN scale sc_g folds into k^.
  - v bias + out BN fold into the V weights / a host-side final bias.

Device program per core (bf16 SBUF, f32 PSUM):
  x (C=128, (w,h)) --K/QE/QO matmuls--> staging qst/qb-parity + kst strips;
  small SBUF->SBUF DMAs restage each group-pair's 9 contraction channels to
  partition base 0 in per-pair column blocks (q128/k128, zero rows are
  memset once per buffer slot on GPSIMD).  Per w: 4 full-128-row matmuls
  (one per group pair, serial, no PE tiling) give S^T (j, g*128+i) in one
  2-bank PSUM tile; one ACT exp -> PT bf16; 8 AV matmuls (lhsT=PT_g,
  rhs=vT_g|ones col) -> (i, g*17+c) with col 16 = softmax denominator.
  vT comes from per-w matmuls with x_w stationary.  Host divides by the
  denominator, adds the final bias, transposes to (op, H, W).
"""

import numpy as np
import ml_dtypes

import concourse.bacc as bacc
import concourse.mybir as mybir
import concourse.tile as tile
from concourse.bass_utils import run_bass_kernel_spmd

BF16 = mybir.dt.bfloat16
F16 = mybir.dt.float16
F32 = mybir.dt.float32
NPBF16 = ml_dtypes.bfloat16
NPF16 = np.float16

N, C, H, W = 8, 128, 128, 128
G = 8
EPS = 1e-5
CHUNK = 16                       # w-columns per pipeline chunk
NCHUNK = W // CHUNK
EXP_SHIFT = -25.0                # uniform exp shift, cancels in num/den

_NC = None
_NC_LOOP = {}


def _build_nc(loop_n=None):
    nc = bacc.Bacc()

    x_d = nc.dram_tensor("x", [C, W * H], F16, kind="ExternalInput")
    qones_d = nc.dram_tensor("qones", [1, CHUNK * 128], F16, kind="ExternalInput")
    wqe_d = nc.dram_tensor("wqe", [C, 128], F16, kind="ExternalInput")
    wqo_d = nc.dram_tensor("wqo", [C, 128], F16, kind="ExternalInput")
    wk_d = nc.dram_tensor("wk", [C, 128], F16, kind="ExternalInput")
    wv_d = nc.dram_tensor("wv", [C, 128], F16, kind="ExternalInput")
    out_d = nc.dram_tensor("out", [H, W * 136], BF16, kind="ExternalOutput")

    with tile.TileContext(nc) as tc:
        with (
            tc.tile_pool(name="wp", bufs=1) as wp,
            tc.tile_pool(name="xp", bufs=2) as xp,
            tc.tile_pool(name="qst", bufs=2) as qstp,
            tc.tile_pool(name="kst", bufs=2) as kstp,
            tc.tile_pool(name="q128", bufs=2) as q128p,
            tc.tile_pool(name="k128", bufs=2) as k128p,
            tc.tile_pool(name="vt", bufs=3) as vtp,
            tc.tile_pool(name="ot", bufs=2) as otp,
            tc.tile_pool(name="pt", bufs=3) as ptp,
            tc.tile_pool(name="psA", bufs=2, space="PSUM") as psA,
            tc.tile_pool(name="psB", bufs=3, space="PSUM") as psB,
            tc.tile_pool(name="psC", bufs=1, space="PSUM") as psC,
        ):
            bias_t = wp.tile([128, 1], F32, tag="ebias")
            nc.vector.memset(bias_t, EXP_SHIFT)
            wqe_t = wp.tile([C, 128], F16, tag="wqe")
            wqo_t = wp.tile([C, 128], F16, tag="wqo")
            wk_t = wp.tile([C, 128], F16, tag="wk")
            wv_t = wp.tile([C, 128], F16, tag="wv")
            nc.sync.dma_start(out=wqe_t, in_=wqe_d[:, :])
            nc.sync.dma_start(out=wqo_t, in_=wqo_d[:, :])
            nc.sync.dma_start(out=wk_t, in_=wk_d[:, :])
            nc.sync.dma_start(out=wv_t, in_=wv_d[:, :])

            def prep_gen(ch, slot):
                """Generator emitting prep work in small quanta so it can be
                interleaved into the previous chunk's attention stream."""
                ctx = tc.high_priority(offset=2000)
                ctx.__enter__()
                x_t = xp.tile([C, CHUNK * 128], F16, tag="x")
                nc.gpsimd.dma_start(
                    out=x_t, in_=x_d[:, ch * CHUNK * 128:(ch + 1) * CHUNK * 128]
                )

                qst_t = qstp.tile([128, CHUNK * 256], F16, tag="qst")
                kst_t = kstp.tile([128, CHUNK * 128], F16, tag="kst")
                q128_t = q128p.tile([128, CHUNK * 1024], F16, tag="q128")
                k128_t = k128p.tile([128, CHUNK * 512], F16, tag="k128")
                vt_t = vtp.tile([128, CHUNK * 136], BF16, tag="vt")
                slot[ch] = (q128_t, k128_t, vt_t)
                yield

                # QKV matmuls over 512-wide (4 w) sub-blocks
                for s in range(CHUNK // 4):
                    rhs = x_t[:, s * 512:(s + 1) * 512]

                    pk = psB.tile([128, 512], F32, tag="ps512")
                    nc.tensor.matmul(pk, lhsT=wk_t, rhs=rhs, start=True, stop=True)
                    nc.vector.tensor_copy(kst_t[:, s * 512:(s + 1) * 512], pk)
                    yield

                    qst_blk = qst_t[:, s * 1024:(s + 1) * 1024].rearrange(
                        "p (w t i) -> p w t i", t=2, i=128
                    )
                    pe = psB.tile([128, 512], F32, tag="ps512")
                    nc.tensor.matmul(pe, lhsT=wqe_t, rhs=rhs, start=True, stop=True)
                    nc.vector.tensor_copy(
                        qst_blk[:, :, 0, :],
                        pe.rearrange("p (w i) -> p w i", i=128),
                    )
                    yield
                    po = psB.tile([128, 512], F32, tag="ps512")
                    nc.tensor.matmul(po, lhsT=wqo_t, rhs=rhs, start=True, stop=True)
                    nc.vector.tensor_copy(
                        qst_blk[:, :, 1, :],
                        po.rearrange("p (w i) -> p w i", i=128),
                    )
                    yield

                # restage each pair's 32-row strip to partition base 0 as a
                # t-major column block — one fully contiguous DMA per pair.
                for t in range(4):
                    nc.sync.dma_start(
                        out=k128_t[0:32, t * CHUNK * 128:(t + 1) * CHUNK * 128],
                        in_=kst_t[32 * t:32 * t + 32, :],
                    )
                    nc.sync.dma_start(
                        out=q128_t[0:32, t * CHUNK * 256:(t + 1) * CHUNK * 256],
                        in_=qst_t[32 * t:32 * t + 32, :],
                    )
                    yield

                # ones rows of q~ via DMA (row 8 even halves / row 24 odd)
                q128_f = q128_t.rearrange(
                    "p (b par i) -> p b par i", par=2, i=128
                )
                ones_in = qones_d[0:1, 0:128].rearrange(
                    "p (a i) -> p a i", a=1
                ).to_broadcast((1, CHUNK * 4, 128))
                nc.sync.dma_start(out=q128_f[8:9, :, 0, :], in_=ones_in)
                nc.sync.dma_start(out=q128_f[24:25, :, 1, :], in_=ones_in)
                yield

                # vT matmuls (x_w stationary), 4 w per PSUM tile
                for s in range(CHUNK // 4):
                    pv = psB.tile([128, 512], F32, tag="ps512")
                    for v in range(4):
                        wl = s * 4 + v
                        nc.tensor.matmul(
                            pv[:, v * 128:(v + 1) * 128],
                            lhsT=x_t[:, wl * 128:(wl + 1) * 128],
                            rhs=wv_t,
                            start=True, stop=True,
                        )
                    nc.vector.tensor_copy(
                        vt_t.rearrange("p (w g c) -> p w g c", g=G, c=17)[
                            :, s * 4:(s + 1) * 4, :, 0:16
                        ],
                        pv.rearrange("p (w g c) -> p w g c", g=G, c=16),
                    )
                    yield
                nc.vector.memset(
                    vt_t.rearrange("p (w g c) -> p w g c", g=G, c=17)[:, :, :, 16],
                    1.0,
                )
                ctx.__exit__(None, None, None)

            def pull(gen, n=1):
                if gen is None:
                    return
                for _ in range(n):
                    try:
                        next(gen)
                    except StopIteration:
                        return

            def attention(ch, state, gen):
                q128_t, k128_t, vt_t = state
                ot_t = otp.tile([128, CHUNK * 136], BF16, tag="ot")

                def emit_av(ptt, wl):
                    o2 = psC.tile([128, 136], F32, tag="o2")
                    for g in range(G):
                        nc.tensor.matmul(
                            o2[:, g * 17:(g + 1) * 17],
                            lhsT=ptt[:, g * 128:(g + 1) * 128],
                            rhs=vt_t[:, wl * 136 + g * 17:wl * 136 + (g + 1) * 17],
                            start=True, stop=True,
                        )
                    nc.vector.tensor_copy(
                        ot_t[:, wl * 136:(wl + 1) * 136], o2[:, 0:136]
                    )

                prev = None
                for wl in range(CHUNK):
                    sp = psA.tile([128, 1024], F32, tag="spsum")
                    for t in range(4):
                        nc.tensor.matmul(
                            sp[:, t * 256:(t + 1) * 256],
                            lhsT=k128_t[0:32, (t * CHUNK + wl) * 128:(t * CHUNK + wl + 1) * 128],
                            rhs=q128_t[0:32, (t * CHUNK + wl) * 256:(t * CHUNK + wl + 1) * 256],
                            start=True, stop=True,
                        )
                    ptt = ptp.tile([128, 1024], BF16, tag="pt")
                    nc.scalar.activation(
                        ptt, sp, mybir.ActivationFunctionType.Exp, bias=bias_t[:, 0:1]
                    )
                    if prev is not None:
                        emit_av(*prev)
                    prev = (ptt, wl)
                    pull(gen, 2)
                emit_av(*prev)

                nc.gpsimd.dma_start(
                    out=out_d[:, ch * CHUNK * 136:(ch + 1) * CHUNK * 136],
                    in_=ot_t,
                )

            # chunk-level software pipeline with fine-grained interleave of
            # the next chunk's prep into this chunk's attention stream.
            def pipeline():
                slot = {}
                g0 = prep_gen(0, slot)
                pull(g0, 100)
                for ch in range(NCHUNK):
                    gen = prep_gen(ch + 1, slot) if ch + 1 < NCHUNK else None
                    pull(gen, 1)
                    attention(ch, slot[ch], gen)
                    pull(gen, 100)

            if loop_n is None:
                pipeline()
            else:
                with tc.For_i(0, loop_n, 1):
                    pipeline()

    return nc


def _get_nc(loop_n=None):
    global _NC
    if loop_n is not None:
        if loop_n not in _NC_LOOP:
            nc = _build_nc(loop_n)
            if not nc.is_finalized():
                nc.finalize()
            _NC_LOOP[loop_n] = nc
        return _NC_LOOP[loop_n]
    if _NC is None:
        _NC = _build_nc()
        if not _NC.is_finalized():
            _NC.finalize()
    return _NC


def _host_prep(x, w_qkv, qkv_gamma, qkv_beta, qkv_mean, qkv_var,
               sim_gamma, sim_var, out_gamma, out_beta, out_mean, out_var):
    s = (qkv_gamma / np.sqrt(qkv_var + EPS)).astype(np.float64)
    b = qkv_beta - qkv_mean * s
    Ws = (w_qkv.astype(np.float64) * s[:, None]).reshape(G, 32, C)
    bs = b.reshape(G, 32)
    Wq, Wk, Wv = Ws[:, 0:8], Ws[:, 8:16], Ws[:, 16:32]
    bq, bv = bs[:, 0:8], bs[:, 16:32]

    sc = sim_gamma / np.sqrt(sim_var + EPS)                     # (G,)
    s2 = out_gamma / np.sqrt(out_var + EPS)                     # (128,) op
    b2 = out_beta - out_mean * s2
    bfinal = (b2 + s2 * bv.reshape(128)).astype(np.float32)

    WQE = np.zeros((128, C), np.float64)
    WQO = np.zeros((128, C), np.float64)
    WKm = np.zeros((128, C), np.float64)
    WVm = np.zeros((128, C), np.float64)
    for t in range(4):
        WQE[32 * t:32 * t + 8] = Wq[2 * t]
        WQO[32 * t + 16:32 * t + 24] = Wq[2 * t + 1]
    for g in range(G):
        WKm[16 * g:16 * g + 8] = sc[g] * Wk[g]
        WKm[16 * g + 8] = sc[g] * (bq[g] @ Wk[g])
        WVm[16 * g:16 * g + 16] = s2[16 * g:16 * g + 16, None] * Wv[g]

    wqe = np.ascontiguousarray(WQE.T).astype(NPF16)             # (C, 128)
    wqo = np.ascontiguousarray(WQO.T).astype(NPF16)
    wk = np.ascontiguousarray(WKm.T).astype(NPF16)
    wv = np.ascontiguousarray(WVm.T).astype(NPF16)

    # x: (N, C, H, W) -> per core (C, W, H) bf16
    xb = np.ascontiguousarray(x.transpose(0, 1, 3, 2)).astype(NPF16)
    xb = xb.reshape(N, C, W * H)
    return xb, wqe, wqo, wk, wv, bfinal


def _run(inputs, trace=False, loop_n=None):
    nc = _get_nc(loop_n)
    xb, wqe, wqo, wk, wv, bfinal = _host_prep(
        inputs["x"], inputs["w_qkv"],
        inputs["qkv_gamma"], inputs["qkv_beta"], inputs["qkv_mean"],
        inputs["qkv_var"], inputs["sim_gamma"], inputs["sim_var"],
        inputs["out_gamma"], inputs["out_beta"], inputs["out_mean"],
        inputs["out_var"],
    )
    qones = np.ones((1, CHUNK * 128), NPF16)
    in_maps = [
        {"x": xb[n], "wqe": wqe, "wqo": wqo, "wk": wk, "wv": wv, "qones": qones}
        for n in range(N)
    ]
    res = run_bass_kernel_spmd(
        nc, in_maps, core_ids=list(range(N)), trace=trace
    )

    outs = np.empty((N, 128, H, W), np.float32)
    for n in range(N):
        raw = res.results[n]["out"].astype(np.float32)
        raw = raw.reshape(H, W, G, 17)
        o = raw[..., :16] / raw[..., 16:17]                     # (H, W, G, 16)
        outs[n] = o.transpose(2, 3, 0, 1).reshape(128, H, W)
    outs += bfinal[None, :, None, None]
    return outs, res


def kernel(x, w_qkv, qkv_gamma, qkv_beta, qkv_mean, qkv_var,
           sim_gamma, sim_beta, sim_mean, sim_var,
           out_gamma, out_beta, out_mean, out_var):
    inputs = dict(
        x=np.asarray(x, np.float32), w_qkv=np.asarray(w_qkv, np.float32),
        qkv_gamma=np.asarray(qkv_gamma, np.float32),
        qkv_beta=np.asarray(qkv_beta, np.float32),
        qkv_mean=np.asarray(qkv_mean, np.float32),
        qkv_var=np.asarray(qkv_var, np.float32),
        sim_gamma=np.asarray(sim_gamma, np.float32),
        sim_var=np.asarray(sim_var, np.float32),
        out_gamma=np.asarray(out_gamma, np.float32),
        out_beta=np.asarray(out_beta, np.float32),
        out_mean=np.asarray(out_mean, np.float32),
        out_var=np.asarray(out_var, np.float32),
    )
    outs, _ = _run(inputs, trace=False)
    return outs
